# revision 28
# baseline (speedup 1.0000x reference)
"""Trainium2 Bass kernel for nn_AttentionBlock (B=4, H=W=64, C=512, Cr=64).

Reference computation (per batch sample b):
    xf = x[b].reshape(N=4096, C=512)
    q = xf @ Wf; k = xf @ Wg; v = xf @ Wh
    attn = softmax(q @ k.T, axis=-1)
    out[b] = gamma * (attn @ v) + x[b]

Sharding: 8 cores, data-parallel over B=4 with 2-way sequence-parallel over
query rows. Core c handles batch c//2, query-row half c%2 (2048 rows).
Each core receives the full 4096x512 x of its batch, permuted so its OWN
2048 query rows come first (softmax over keys is permutation invariant as
long as k and v use the same key order, which they do). The program is
identical on all cores (SPMD); only the input data differs.

Host-side marshaling pre-packs every input into the exact SBUF layout
(partition-major, KC-chunked) so each tensor loads with ONE wide DMA.

Per-core dataflow:
  1. qT = Wf.T @ xT (bf16); kT = Wg.T @ xT (bf16, duplicated onto
     partitions 64..127 for PE-array row-packing of the score matmuls);
     v = (16x) @ (16Wh) in fp8e4 DoubleRow, stored as fp8 pair tiles
     v8p[kt/2][128, 2, 512] = 16*v.
  2. Per 512-row block, two passes over the 32 key tiles:
     P1: packed score matmuls (two 64-contraction matmuls on disjoint PE
         halves) -> exp on ScalarE (bf16, no max subtraction: scores are
         fp32/bf16-safe) into a 20-deep e1 ring; row sums accumulate as a
         TRANSPOSED [1, 512] psum row via ones.T @ e1 matmuls (1-column
         stationary => no LDWEIGHTS cost).
     F:  F_row = 1/Z on VectorE, broadcast to all 128 partitions on
         GpSimd (partition_broadcast).
     P2: p8 = e1 * F on VectorE quantized to fp8e4 (p8 <= ~1, exactly
         softmax rows); o += p8.T @ v8 as fp8 DoubleRow matmuls (2 key
         tiles per matmul, 2x PE throughput).
     P1 of block b+1 is interleaved pair-by-pair with P2 of block b so
     ScalarE (exp) and PE stay concurrently busy; the first block's P1
     interleaves with the second half of the v projection.
  3. out = o * (gamma/16) + x fused on VectorE (the /16 compensates the
     fp8 prescales: p8 @ (16 v) = 16 * attn@v), per-row-chunk DMA out.
"""

import sys

if "/opt/trn_rl_repo" not in sys.path:
    sys.path.insert(0, "/opt/trn_rl_repo")

import numpy as np

_BUILt = {}

B, H, W, C = 4, 64, 64, 512
CR = 64          # C // reduction ratio
N = H * W        # 4096 keys per batch
R = N // 2       # 2048 query rows per core
NCORES = 8
NKT = N // 128   # 32 key tiles
NKP = NKT // 2   # 16 key-tile pairs
KC = C // 128    # 4 contraction chunks over C


def _build():
    import ml_dtypes
    import concourse.bass as bass
    import concourse.mybir as mybir
    import concourse.tile as tile
    from concourse import bacc

    f32 = mybir.dt.float32
    bf16 = mybir.dt.bfloat16
    fp8 = mybir.dt.float8e4
    Exp = mybir.ActivationFunctionType.Exp
    mult = mybir.AluOpType.mult
    add = mybir.AluOpType.add
    DR = mybir.MatmulPerfMode.DoubleRow

    nc = bacc.Bacc(
        "TRN2",
        target_bir_lowering=False,
        debug=False,
        num_devices=NCORES,
    )

    xta_d = nc.dram_tensor("xta", [128, KC, R], bf16, kind="ExternalInput")
    xtb_d = nc.dram_tensor("xtb", [128, KC, R], bf16, kind="ExternalInput")
    x8a_d = nc.dram_tensor("x8a", [128, KC, R], fp8, kind="ExternalInput")
    x8b_d = nc.dram_tensor("x8b", [128, KC, R], fp8, kind="ExternalInput")
    wf_d = nc.dram_tensor("wf", [128, KC, CR], bf16, kind="ExternalInput")
    wg_d = nc.dram_tensor("wg", [128, KC, CR], bf16, kind="ExternalInput")
    wh8_d = nc.dram_tensor("wh8", [128, KC, C], fp8, kind="ExternalInput")
    gam_d = nc.dram_tensor("gammav", [128, 1], f32, kind="ExternalInput")
    xres_d = nc.dram_tensor("xres", [128, 16, C], bf16, kind="ExternalInput")
    out_d = nc.dram_tensor("out", [128, 16, C], f32, kind="ExternalOutput")

    ones_d = nc.inline_tensor(
        np.ones((128, 1), dtype=ml_dtypes.bfloat16), name="onesc"
    )

    with tile.TileContext(nc) as tc:
        with (
            tc.tile_pool(name="const", bufs=1) as cpool,
            tc.tile_pool(name="stand", bufs=1) as spool,
            tc.tile_pool(name="e1", bufs=20) as e1_pool,
            tc.tile_pool(name="p8", bufs=3) as p8_pool,
            tc.tile_pool(name="fpool", bufs=2) as f_pool,
            tc.tile_pool(name="small", bufs=8) as sm_pool,
            tc.tile_pool(name="outp", bufs=4) as out_pool,
        ):
            ones_sb = cpool.tile([128, 1], bf16, name="ones_sb")
            gam_sb = cpool.tile([128, 1], f32, name="gam_sb")
            wf_sb = cpool.tile([128, KC, CR], bf16, name="wf_sb")
            wg_sb = cpool.tile([128, KC, CR], bf16, name="wg_sb")
            wh8_sb = cpool.tile([128, KC, C], fp8, name="wh8_sb")

            xta_sb = spool.tile([128, KC, R], bf16, name="xta_sb")
            xtb_sb = spool.tile([128, KC, R], bf16, name="xtb_sb")
            x8a_sb = spool.tile([128, KC, R], fp8, name="x8a_sb")
            x8b_sb = spool.tile([128, KC, R], fp8, name="x8b_sb")
            xres_sb = spool.tile([128, 16, C], bf16, name="xres_sb")
            v8p = [
                spool.tile([128, 2, C], fp8, name=f"v8p{t}")
                for t in range(NKP)
            ]
            kTd = spool.tile([128, N], bf16, name="kTd")
            qTd = spool.tile([128, R], bf16, name="qTd")

            # ---- one wide DMA per tensor; q projection unblocks first ----
            nc.sync.dma_start(out=wf_sb[:], in_=wf_d[:])
            for kc in range(KC):
                nc.sync.dma_start(
                    out=xta_sb[:, kc, :], in_=xta_d[:, kc, :]
                )
            nc.sync.dma_start(out=wg_sb[:], in_=wg_d[:])
            nc.sync.dma_start(out=xtb_sb[:], in_=xtb_d[:])
            nc.sync.dma_start(out=wh8_sb[:], in_=wh8_d[:])
            nc.sync.dma_start(out=x8a_sb[:], in_=x8a_d[:])
            nc.sync.dma_start(out=x8b_sb[:], in_=x8b_d[:])
            nc.sync.dma_start(out=ones_sb[:], in_=ones_d[:])
            nc.sync.dma_start(out=gam_sb[:], in_=gam_d[:])
            nc.sync.dma_start(out=xres_sb[:], in_=xres_d[:])

            with tc.tile_pool(name="psc", bufs=1, space="PSUM") as p3:
                saved_e1 = [[None] * NKP for _ in range(4)]
                f_tiles = [None] * 4
                o_cur = [None]
                o_pool = [None]

                zt_cur = [None]

                def emit_zt(blk, p, zt):
                    e1p = saved_e1[blk][p]
                    for sub in range(2):
                        nc.tensor.matmul(
                            zt[:],
                            lhsT=ones_sb[:],
                            rhs=e1p[:, sub, :],
                            start=(p == 0 and sub == 0),
                            stop=(p == NKP - 1 and sub == 1),
                            skip_group_check=True,
                        )

                def emit_p1_pair(blk, ktp):
                    # zt (row sums) for pair ktp-2 is emitted here, two
                    # pairs behind the exp that feeds it: the PE is
                    # in-order, so a zt matmul gated on a just-issued exp
                    # would stall the PE (and everything queued after it)
                    # on ScalarE for ~half the exp latency every pair.
                    if ktp == 0:
                        zt_cur[0] = p3.tile(
                            [1, 512], f32, tag="zt", name=f"zt{blk}"
                        )
                    e1p = e1_pool.tile(
                        [128, 2, 512], bf16, tag="e1", name="e1"
                    )
                    saved_e1[blk][ktp] = e1p
                    for sub in range(2):
                        kt = 2 * ktp + sub
                        hp = sub * CR
                        sch = p3.tile(
                            [128, 512], f32, tag="sc", bufs=3, name="sch"
                        )
                        nc.tensor.matmul(
                            sch[:],
                            lhsT=kTd[
                                hp : hp + CR, kt * 128 : (kt + 1) * 128
                            ],
                            rhs=qTd[
                                hp : hp + CR, blk * 512 : (blk + 1) * 512
                            ],
                            start=True,
                            stop=True,
                        )
                        nc.scalar.activation(
                            e1p[:, sub, :], sch[:], Exp
                        )
                    if ktp >= 2:
                        emit_zt(blk, ktp - 2, zt_cur[0])
                    if ktp == NKP - 1:
                        emit_zt(blk, ktp - 1, zt_cur[0])
                        emit_zt(blk, ktp, zt_cur[0])
                    return zt_cur[0]

                def emit_f(blk, zt):
                    frow = sm_pool.tile(
                        [1, 512], bf16, tag="frow", name="frow"
                    )
                    with nc.allow_low_precision(
                        reason="1/Z at bf16: 0.4% row-scale noise, far "
                        "under the fp8e4 P quantization already accepted"
                    ):
                        nc.vector.reciprocal(frow[:], zt[:])
                    fsb = f_pool.tile(
                        [128, 2, 512], bf16, tag="fsb", name="fsb"
                    )
                    for sub in range(2):
                        nc.gpsimd.partition_broadcast(
                            fsb[:, sub, :], frow[0:1, :]
                        )
                    f_tiles[blk] = fsb

                def emit_p2_pair(blk, ktp):
                    if ktp == 0:
                        o_cur[0] = [
                            o_pool[0].tile(
                                [128, C], f32, tag=f"o{rc}",
                                name=f"ops{blk}_{rc}",
                            )
                            for rc in range(4)
                        ]
                    e1p = saved_e1[blk][ktp]
                    saved_e1[blk][ktp] = None
                    p8t = p8_pool.tile(
                        [128, 2, 512], fp8, tag="p8", name="p8"
                    )
                    nc.vector.tensor_mul(
                        p8t[:, :, :], e1p[:, :, :], f_tiles[blk][:, :, :]
                    )
                    for rc in range(4):
                        nc.tensor.matmul(
                            o_cur[0][rc][:],
                            lhsT=p8t[:, :, rc * 128 : (rc + 1) * 128],
                            rhs=v8p[ktp][:],
                            start=(ktp == 0),
                            stop=(ktp == NKP - 1),
                            perf_mode=DR,
                        )

                def emit_epilogue(blk):
                    for rc in range(4):
                        ot = out_pool.tile([128, C], f32, tag="ot", name="ot")
                        nc.vector.scalar_tensor_tensor(
                            out=ot[:],
                            in0=o_cur[0][rc][:],
                            scalar=gam_sb[:],
                            in1=xres_sb[:, blk * 4 + rc, :],
                            op0=mult,
                            op1=add,
                        )
                        nc.sync.dma_start(
                            out=out_d[:, blk * 4 + rc, :], in_=ot[:]
                        )

                def emit_kq(w_sb, dst_sb, xT, nt_local, dst_off, kq_pool,
                            on_vector=False):
                    ps = kq_pool.tile([CR, 512], f32, tag="kq", name="kqp")
                    for kc in range(KC):
                        nc.tensor.matmul(
                            ps[:],
                            lhsT=w_sb[:, kc, :],
                            rhs=xT[:, kc, nt_local * 512 : (nt_local + 1) * 512],
                            start=(kc == 0),
                            stop=(kc == KC - 1),
                        )
                    dst = dst_sb[0:CR, dst_off : dst_off + 512]
                    if on_vector:
                        nc.vector.tensor_copy(dst, ps[:])
                    else:
                        nc.scalar.copy(dst, ps[:])

                def emit_v(x8, kt, vps_pool, on_scalar=False):
                    kt16 = kt % 16
                    sl = slice(kt16 * 128, (kt16 + 1) * 128)
                    ps = vps_pool.tile([128, C], f32, tag="vps", name="vp")
                    nc.tensor.matmul(
                        ps[:],
                        lhsT=x8[:, 0:2, sl],
                        rhs=wh8_sb[:, 0:2, :],
                        start=True,
                        stop=False,
                        perf_mode=DR,
                    )
                    nc.tensor.matmul(
                        ps[:],
                        lhsT=x8[:, 2:4, sl],
                        rhs=wh8_sb[:, 2:4, :],
                        start=False,
                        stop=True,
                        perf_mode=DR,
                    )
                    dst = v8p[kt // 2][:, kt % 2, :]
                    if on_scalar:
                        nc.scalar.activation(
                            dst, ps[:], mybir.ActivationFunctionType.Copy,
                            scale=0.0625,
                        )
                    else:
                        nc.vector.tensor_scalar_mul(dst, ps[:], 0.0625)

                # ---- phase 2: projections (q, k fully; v first half) ----
                with tc.tile_pool(name="ps2kq", bufs=2, space="PSUM") as kqp:
                    for nt in range(4):
                        emit_kq(wf_sb, qTd, xta_sb, nt, nt * 512, kqp)
                    nc.sync.dma_start(out=qTd[CR:128, :], in_=qTd[0:CR, :])
                    for nt in range(4):
                        emit_kq(wg_sb, kTd, xta_sb, nt, nt * 512, kqp)
                    # split the kT h64-duplication per half so block 0's
                    # first 8 score pairs unblock right after k-a
                    nc.sync.dma_start(
                        out=kTd[CR:128, 0:R], in_=kTd[0:CR, 0:R]
                    )
                    for nt in range(4):
                        emit_kq(wg_sb, kTd, xtb_sb, nt, 2048 + nt * 512, kqp)
                    nc.sync.dma_start(
                        out=kTd[CR:128, R:N], in_=kTd[0:CR, R:N]
                    )

                with tc.tile_pool(name="ps2v", bufs=2, space="PSUM") as vps:
                    # ---- head slot: P1(0) leads; exp-paced scores leave
                    # ~38% PE slack, so the v-a matmuls drip in from pair 6
                    # onwards (by then the x8a DMA has landed — a v matmul
                    # stalled on DMA in the in-order PE queue would block
                    # every score behind it). v-b runs at the head's end.
                    # All v copies on DVE: ScalarE is exp-saturated here. ----
                    va_queue = list(range(16))
                    for p in range(NKP):
                        zt0 = emit_p1_pair(0, p)
                        if p >= 6:
                            for _ in range(2 if p >= 8 else 1):
                                if va_queue:
                                    emit_v(x8a_sb, va_queue.pop(0), vps)
                    for kt in range(16, 32):
                        emit_v(x8b_sb, kt, vps)

                emit_f(0, zt0)
                with tc.tile_pool(name="po", bufs=1, space="PSUM") as pop:
                    o_pool[0] = pop
                    # ---- mid slots: P1(b+1) pair-interleaved with P2(b),
                    # P1 LEADING by 2 pairs: the PE queue is in-order, so
                    # o-matmuls emitted between a score and the exp-gated
                    # reuse of its psum slot would put the o latency inside
                    # the score->exp->ring feedback loop ----
                    LEAD = 2
                    for b in range(3):
                        for p in range(NKP + LEAD):
                            if p < NKP:
                                ztn = emit_p1_pair(b + 1, p)
                            if p >= LEAD:
                                emit_p2_pair(b, p - LEAD)
                        emit_f(b + 1, ztn)
                        emit_epilogue(b)
                    # ---- tail slot ----
                    for p in range(NKP):
                        emit_p2_pair(3, p)
                    emit_epilogue(3)

    nc.compile()
    return nc


def _get_nc():
    if "nc" not in _BUILt:
        _BUILt["nc"] = _build()
    return _BUILt["nc"]


def make_in_maps(x, Wf, Wg, Wh, gamma):
    import ml_dtypes

    bf16 = ml_dtypes.bfloat16
    fp8 = ml_dtypes.float8_e4m3

    def chunkp(a, d):
        # [KC*128, d] -> [128, KC, d] partition-major
        return np.ascontiguousarray(
            a.reshape(KC, 128, d).transpose(1, 0, 2)
        )

    x = np.asarray(x, dtype=np.float32)
    gv = np.full(
        (128, 1), np.float32(np.asarray(gamma).reshape(-1)[0]) / 16.0,
        dtype=np.float32,
    )
    wf = chunkp(np.asarray(Wf, np.float32).astype(bf16), CR)
    wg = chunkp(np.asarray(Wg, np.float32).astype(bf16), CR)
    wh8 = chunkp((np.asarray(Wh, np.float32) * 16.0).astype(fp8), C)
    in_maps = []
    for core in range(NCORES):
        b, h = divmod(core, 2)
        xb = x[b].reshape(N, C)
        own = xb[h * R : (h + 1) * R]
        other = xb[(1 - h) * R : (2 - h) * R]
        xp = np.concatenate([own, other], axis=0)
        xpT = xp.T  # [C, N]
        xt = chunkp(xpT.astype(bf16), N)
        x8 = chunkp((xpT * 16.0).astype(fp8), N)
        xres = np.ascontiguousarray(
            own.reshape(16, 128, C).transpose(1, 0, 2).astype(bf16)
        )
        in_maps.append(
            {
                "xta": np.ascontiguousarray(xt[:, :, 0:R]),
                "xtb": np.ascontiguousarray(xt[:, :, R:N]),
                "x8a": np.ascontiguousarray(x8[:, :, 0:R]),
                "x8b": np.ascontiguousarray(x8[:, :, R:N]),
                "wf": wf,
                "wg": wg,
                "wh8": wh8,
                "gammav": gv,
                "xres": xres,
            }
        )
    return in_maps


def gather_out(results, x):
    out = np.empty((B, N, C), dtype=np.float32)
    for core in range(NCORES):
        b, h = divmod(core, 2)
        o = results[core]["out"].transpose(1, 0, 2).reshape(R, C)
        out[b, h * R : (h + 1) * R] = o
    return out.reshape(B, H, W, C)


def run(x, Wf, Wg, Wh, gamma, **spmd_kwargs):
    from concourse.bass_utils import run_bass_kernel_spmd

    nc = _get_nc()
    in_maps = make_in_maps(x, Wf, Wg, Wh, gamma)
    res = run_bass_kernel_spmd(
        nc, in_maps, core_ids=list(range(NCORES)), **spmd_kwargs
    )
    return gather_out(res.results, x), res


def kernel(x, Wf, Wg, Wh, gamma):
    out, _ = run(x, Wf, Wg, Wh, gamma)
    return out


# revision 30
# speedup vs baseline: 1.2248x; 1.2248x over previous
"""Trainium2 Bass kernel for nn_AttentionBlock (B=4, H=W=64, C=512, Cr=64).

Reference computation (per batch sample b):
    xf = x[b].reshape(N=4096, C=512)
    q = xf @ Wf; k = xf @ Wg; v = xf @ Wh
    attn = softmax(q @ k.T, axis=-1)
    out[b] = gamma * (attn @ v) + x[b]

Sharding: 8 cores, data-parallel over B=4 with 2-way sequence-parallel over
query rows. Core c handles batch c//2, query-row half c%2 (2048 rows).
Each core receives the full 4096x512 x of its batch, permuted so its OWN
2048 query rows come first (softmax over keys is permutation invariant as
long as k and v use the same key order, which they do). The program is
identical on all cores (SPMD); only the input data differs.

Host-side marshaling pre-packs every input into the exact SBUF layout
(partition-major, KC-chunked) so each tensor loads with ONE wide DMA.

Per-core dataflow:
  1. qT = Wf.T @ xT (bf16); kT = Wg.T @ xT (bf16, duplicated onto
     partitions 64..127 for PE-array row-packing of the score matmuls);
     v = (16x) @ (16Wh) in fp8e4 DoubleRow, stored as fp8 pair tiles
     v8p[kt/2][128, 2, 512] = 16*v.
  2. Per 512-row block, two passes over the 32 key tiles:
     P1: packed score matmuls (two 64-contraction matmuls on disjoint PE
         halves) -> exp on ScalarE (bf16, no max subtraction: scores are
         fp32/bf16-safe) into a 20-deep e1 ring; row sums accumulate as a
         TRANSPOSED [1, 512] psum row via ones.T @ e1 matmuls (1-column
         stationary => no LDWEIGHTS cost).
     F:  F_row = 1/Z on VectorE, broadcast to all 128 partitions on
         GpSimd (partition_broadcast).
     P2: p8 = e1 * F on VectorE quantized to fp8e4 (p8 <= ~1, exactly
         softmax rows); o += p8.T @ v8 as fp8 DoubleRow matmuls (2 key
         tiles per matmul, 2x PE throughput).
     P1 of block b+1 is interleaved pair-by-pair with P2 of block b so
     ScalarE (exp) and PE stay concurrently busy; the first block's P1
     interleaves with the second half of the v projection.
  3. out = o * (gamma/16) + x fused on VectorE (the /16 compensates the
     fp8 prescales: p8 @ (16 v) = 16 * attn@v), per-row-chunk DMA out.
"""

import sys

if "/opt/trn_rl_repo" not in sys.path:
    sys.path.insert(0, "/opt/trn_rl_repo")

import numpy as np

_BUILt = {}

B, H, W, C = 4, 64, 64, 512
CR = 64          # C // reduction ratio
N = H * W        # 4096 keys per batch
R = N // 2       # 2048 query rows per core
NCORES = 8
NKT = N // 128   # 32 key tiles
NKP = NKT // 2   # 16 key-tile pairs
KC = C // 128    # 4 contraction chunks over C


def _build():
    import ml_dtypes
    import concourse.bass as bass
    import concourse.mybir as mybir
    import concourse.tile as tile
    from concourse import bacc

    f32 = mybir.dt.float32
    bf16 = mybir.dt.bfloat16
    fp8 = mybir.dt.float8e4
    Exp = mybir.ActivationFunctionType.Exp
    mult = mybir.AluOpType.mult
    add = mybir.AluOpType.add
    DR = mybir.MatmulPerfMode.DoubleRow

    nc = bacc.Bacc(
        "TRN2",
        target_bir_lowering=False,
        debug=False,
        num_devices=NCORES,
    )

    xta_d = nc.dram_tensor("xta", [128, KC, R], bf16, kind="ExternalInput")
    xtb_d = nc.dram_tensor("xtb", [128, KC, R], bf16, kind="ExternalInput")
    x8a_d = nc.dram_tensor("x8a", [128, KC, R], fp8, kind="ExternalInput")
    x8b_d = nc.dram_tensor("x8b", [128, KC, R], fp8, kind="ExternalInput")
    wf_d = nc.dram_tensor("wf", [128, KC, CR], bf16, kind="ExternalInput")
    wg_d = nc.dram_tensor("wg", [128, KC, CR], bf16, kind="ExternalInput")
    wh8_d = nc.dram_tensor("wh8", [128, KC, C], fp8, kind="ExternalInput")
    gam_d = nc.dram_tensor("gammav", [128, 1], f32, kind="ExternalInput")
    xres_d = nc.dram_tensor("xres", [128, 16, C], bf16, kind="ExternalInput")
    out_d = nc.dram_tensor("out", [128, 16, C], f32, kind="ExternalOutput")

    ones_d = nc.inline_tensor(
        np.ones((128, 1), dtype=ml_dtypes.bfloat16), name="onesc"
    )

    with tile.TileContext(nc) as tc:
        with (
            tc.tile_pool(name="const", bufs=1) as cpool,
            tc.tile_pool(name="stand", bufs=1) as spool,
            tc.tile_pool(name="e1", bufs=22) as e1_pool,
            tc.tile_pool(name="p8", bufs=4) as p8_pool,
            tc.tile_pool(name="fpool", bufs=2) as f_pool,
            tc.tile_pool(name="small", bufs=8) as sm_pool,
            tc.tile_pool(name="outp", bufs=4) as out_pool,
        ):
            ones_sb = cpool.tile([128, 1], bf16, name="ones_sb")
            gam_sb = cpool.tile([128, 1], f32, name="gam_sb")
            wf_sb = cpool.tile([128, KC, CR], bf16, name="wf_sb")
            wg_sb = cpool.tile([128, KC, CR], bf16, name="wg_sb")
            wh8_sb = cpool.tile([128, KC, C], fp8, name="wh8_sb")

            xta_sb = spool.tile([128, KC, R], bf16, name="xta_sb")
            xtb_sb = spool.tile([128, KC, R], bf16, name="xtb_sb")
            x8a_sb = spool.tile([128, KC, R], fp8, name="x8a_sb")
            x8b_sb = spool.tile([128, KC, R], fp8, name="x8b_sb")
            xres_sb = spool.tile([128, 16, C], bf16, name="xres_sb")
            v8p = [
                spool.tile([128, 2, C], fp8, name=f"v8p{t}")
                for t in range(NKP)
            ]
            kTd = spool.tile([128, N], bf16, name="kTd")
            qTd = spool.tile([128, R], bf16, name="qTd")

            # ---- one wide DMA per tensor; q projection unblocks first ----
            nc.sync.dma_start(out=wf_sb[:], in_=wf_d[:])
            for kc in range(KC):
                nc.sync.dma_start(
                    out=xta_sb[:, kc, :], in_=xta_d[:, kc, :]
                )
            nc.sync.dma_start(out=wg_sb[:], in_=wg_d[:])
            nc.sync.dma_start(out=xtb_sb[:], in_=xtb_d[:])
            nc.sync.dma_start(out=wh8_sb[:], in_=wh8_d[:])
            nc.sync.dma_start(out=x8a_sb[:], in_=x8a_d[:])
            nc.sync.dma_start(out=x8b_sb[:], in_=x8b_d[:])
            nc.sync.dma_start(out=ones_sb[:], in_=ones_d[:])
            nc.sync.dma_start(out=gam_sb[:], in_=gam_d[:])
            nc.sync.dma_start(out=xres_sb[:], in_=xres_d[:])

            with tc.tile_pool(name="psc", bufs=1, space="PSUM") as p3:
                saved_e1 = [[None] * NKP for _ in range(4)]
                f_tiles = [None] * 4
                o_cur = [None]
                o_pool = [None]

                zt_cur = [None]

                def emit_zt(blk, p, zt):
                    e1p = saved_e1[blk][p]
                    for sub in range(2):
                        nc.tensor.matmul(
                            zt[:],
                            lhsT=ones_sb[:],
                            rhs=e1p[:, sub, :],
                            start=(p == 0 and sub == 0),
                            stop=(p == NKP - 1 and sub == 1),
                            skip_group_check=True,
                        )

                def emit_p1_pair(blk, ktp):
                    # zt (row sums) for pair ktp-2 is emitted here, two
                    # pairs behind the exp that feeds it: the PE is
                    # in-order, so a zt matmul gated on a just-issued exp
                    # would stall the PE (and everything queued after it)
                    # on ScalarE for ~half the exp latency every pair.
                    if ktp == 0:
                        zt_cur[0] = p3.tile(
                            [1, 512], f32, tag="zt", name=f"zt{blk}"
                        )
                    e1p = e1_pool.tile(
                        [128, 2, 512], bf16, tag="e1", name="e1"
                    )
                    saved_e1[blk][ktp] = e1p
                    for sub in range(2):
                        kt = 2 * ktp + sub
                        hp = sub * CR
                        sch = p3.tile(
                            [128, 512], f32, tag="sc", bufs=3, name="sch"
                        )
                        nc.tensor.matmul(
                            sch[:],
                            lhsT=kTd[
                                hp : hp + CR, kt * 128 : (kt + 1) * 128
                            ],
                            rhs=qTd[
                                hp : hp + CR, blk * 512 : (blk + 1) * 512
                            ],
                            start=True,
                            stop=True,
                        )
                        nc.scalar.activation(
                            e1p[:, sub, :], sch[:], Exp
                        )
                    if ktp >= 2:
                        emit_zt(blk, ktp - 2, zt_cur[0])
                    if ktp == NKP - 1:
                        emit_zt(blk, ktp - 1, zt_cur[0])
                        emit_zt(blk, ktp, zt_cur[0])
                    return zt_cur[0]

                def emit_f(blk, zt):
                    frow = sm_pool.tile(
                        [1, 512], bf16, tag="frow", name="frow"
                    )
                    with nc.allow_low_precision(
                        reason="1/Z at bf16: 0.4% row-scale noise, far "
                        "under the fp8e4 P quantization already accepted"
                    ):
                        nc.vector.reciprocal(frow[:], zt[:])
                    fsb = f_pool.tile(
                        [128, 2, 512], bf16, tag="fsb", name="fsb"
                    )
                    for sub in range(2):
                        nc.gpsimd.partition_broadcast(
                            fsb[:, sub, :], frow[0:1, :]
                        )
                    f_tiles[blk] = fsb

                def emit_p2_pair(blk, ktp):
                    if ktp == 0:
                        o_cur[0] = [
                            o_pool[0].tile(
                                [128, C], f32, tag=f"o{rc}",
                                name=f"ops{blk}_{rc}",
                            )
                            for rc in range(4)
                        ]
                    e1p = saved_e1[blk][ktp]
                    saved_e1[blk][ktp] = None
                    p8t = p8_pool.tile(
                        [128, 2, 512], fp8, tag="p8", name="p8"
                    )
                    nc.vector.tensor_mul(
                        p8t[:, :, :], e1p[:, :, :], f_tiles[blk][:, :, :]
                    )
                    for rc in range(4):
                        nc.tensor.matmul(
                            o_cur[0][rc][:],
                            lhsT=p8t[:, :, rc * 128 : (rc + 1) * 128],
                            rhs=v8p[ktp][:],
                            start=(ktp == 0),
                            stop=(ktp == NKP - 1),
                            perf_mode=DR,
                        )

                def emit_epilogue(blk):
                    for rc in range(4):
                        ot = out_pool.tile([128, C], f32, tag="ot", name="ot")
                        nc.vector.scalar_tensor_tensor(
                            out=ot[:],
                            in0=o_cur[0][rc][:],
                            scalar=gam_sb[:],
                            in1=xres_sb[:, blk * 4 + rc, :],
                            op0=mult,
                            op1=add,
                        )
                        nc.sync.dma_start(
                            out=out_d[:, blk * 4 + rc, :], in_=ot[:]
                        )

                def emit_kq(w_sb, dst_sb, xT, nt_local, dst_off, kq_pool,
                            on_vector=False):
                    ps = kq_pool.tile([CR, 512], f32, tag="kq", name="kqp")
                    for kc in range(KC):
                        nc.tensor.matmul(
                            ps[:],
                            lhsT=w_sb[:, kc, :],
                            rhs=xT[:, kc, nt_local * 512 : (nt_local + 1) * 512],
                            start=(kc == 0),
                            stop=(kc == KC - 1),
                        )
                    dst = dst_sb[0:CR, dst_off : dst_off + 512]
                    if on_vector:
                        nc.vector.tensor_copy(dst, ps[:])
                    else:
                        nc.scalar.copy(dst, ps[:])

                def emit_v(x8, kt, vps_pool, on_scalar=False):
                    kt16 = kt % 16
                    sl = slice(kt16 * 128, (kt16 + 1) * 128)
                    ps = vps_pool.tile([128, C], f32, tag="vps", name="vp")
                    nc.tensor.matmul(
                        ps[:],
                        lhsT=x8[:, 0:2, sl],
                        rhs=wh8_sb[:, 0:2, :],
                        start=True,
                        stop=False,
                        perf_mode=DR,
                    )
                    nc.tensor.matmul(
                        ps[:],
                        lhsT=x8[:, 2:4, sl],
                        rhs=wh8_sb[:, 2:4, :],
                        start=False,
                        stop=True,
                        perf_mode=DR,
                    )
                    dst = v8p[kt // 2][:, kt % 2, :]
                    if on_scalar:
                        nc.scalar.activation(
                            dst, ps[:], mybir.ActivationFunctionType.Copy,
                            scale=0.0625,
                        )
                    else:
                        nc.vector.tensor_scalar_mul(dst, ps[:], 0.0625)

                # ---- phase 2: projections (q, k fully; v first half) ----
                with tc.tile_pool(name="ps2kq", bufs=3, space="PSUM") as kqp:
                    for nt in range(4):
                        emit_kq(wf_sb, qTd, xta_sb, nt, nt * 512, kqp,
                                on_vector=(nt % 2 == 1))
                    nc.sync.dma_start(out=qTd[CR:128, :], in_=qTd[0:CR, :])
                    for nt in range(4):
                        emit_kq(wg_sb, kTd, xta_sb, nt, nt * 512, kqp,
                                on_vector=(nt % 2 == 1))
                    # split the kT h64-duplication per half so block 0's
                    # first 8 score pairs unblock right after k-a
                    nc.sync.dma_start(
                        out=kTd[CR:128, 0:R], in_=kTd[0:CR, 0:R]
                    )
                    for nt in range(4):
                        emit_kq(wg_sb, kTd, xtb_sb, nt, 2048 + nt * 512, kqp,
                                on_vector=(nt % 2 == 1))
                    nc.sync.dma_start(
                        out=kTd[CR:128, R:N], in_=kTd[0:CR, R:N]
                    )

                with tc.tile_pool(name="ps2v", bufs=4, space="PSUM") as vps:
                    for kt in range(16):
                        emit_v(x8a_sb, kt, vps, on_scalar=(kt % 2 == 1))
                    # ---- head slot: v second half interleaved with P1 of
                    # block 0 (scores need full kT; v8b tiles are consumed
                    # only from P2 of block 0 onwards) ----
                    for p in range(NKP):
                        emit_v(x8b_sb, 16 + p, vps)
                        zt0 = emit_p1_pair(0, p)

                emit_f(0, zt0)
                with tc.tile_pool(name="po", bufs=1, space="PSUM") as pop:
                    o_pool[0] = pop
                    # ---- mid slots: P1(b+1) pair-interleaved with P2(b),
                    # P1 LEADING by 2 pairs: the PE queue is in-order, so
                    # o-matmuls emitted between a score and the exp-gated
                    # reuse of its psum slot would put the o latency inside
                    # the score->exp->ring feedback loop ----
                    LEAD = 2
                    for b in range(3):
                        for p in range(NKP + LEAD):
                            if p < NKP:
                                ztn = emit_p1_pair(b + 1, p)
                            if p >= LEAD:
                                emit_p2_pair(b, p - LEAD)
                        emit_f(b + 1, ztn)
                        emit_epilogue(b)
                    # ---- tail slot ----
                    for p in range(NKP):
                        emit_p2_pair(3, p)
                    emit_epilogue(3)

    nc.compile()
    return nc


def _get_nc():
    if "nc" not in _BUILt:
        _BUILt["nc"] = _build()
    return _BUILt["nc"]


def make_in_maps(x, Wf, Wg, Wh, gamma):
    import ml_dtypes

    bf16 = ml_dtypes.bfloat16
    fp8 = ml_dtypes.float8_e4m3

    def chunkp(a, d):
        # [KC*128, d] -> [128, KC, d] partition-major
        return np.ascontiguousarray(
            a.reshape(KC, 128, d).transpose(1, 0, 2)
        )

    x = np.asarray(x, dtype=np.float32)
    gv = np.full(
        (128, 1), np.float32(np.asarray(gamma).reshape(-1)[0]) / 16.0,
        dtype=np.float32,
    )
    wf = chunkp(np.asarray(Wf, np.float32).astype(bf16), CR)
    wg = chunkp(np.asarray(Wg, np.float32).astype(bf16), CR)
    wh8 = chunkp((np.asarray(Wh, np.float32) * 16.0).astype(fp8), C)
    in_maps = []
    for core in range(NCORES):
        b, h = divmod(core, 2)
        xb = x[b].reshape(N, C)
        own = xb[h * R : (h + 1) * R]
        other = xb[(1 - h) * R : (2 - h) * R]
        xp = np.concatenate([own, other], axis=0)
        xpT = xp.T  # [C, N]
        xt = chunkp(xpT.astype(bf16), N)
        x8 = chunkp((xpT * 16.0).astype(fp8), N)
        xres = np.ascontiguousarray(
            own.reshape(16, 128, C).transpose(1, 0, 2).astype(bf16)
        )
        in_maps.append(
            {
                "xta": np.ascontiguousarray(xt[:, :, 0:R]),
                "xtb": np.ascontiguousarray(xt[:, :, R:N]),
                "x8a": np.ascontiguousarray(x8[:, :, 0:R]),
                "x8b": np.ascontiguousarray(x8[:, :, R:N]),
                "wf": wf,
                "wg": wg,
                "wh8": wh8,
                "gammav": gv,
                "xres": xres,
            }
        )
    return in_maps


def gather_out(results, x):
    out = np.empty((B, N, C), dtype=np.float32)
    for core in range(NCORES):
        b, h = divmod(core, 2)
        o = results[core]["out"].transpose(1, 0, 2).reshape(R, C)
        out[b, h * R : (h + 1) * R] = o
    return out.reshape(B, H, W, C)


def run(x, Wf, Wg, Wh, gamma, **spmd_kwargs):
    from concourse.bass_utils import run_bass_kernel_spmd

    nc = _get_nc()
    in_maps = make_in_maps(x, Wf, Wg, Wh, gamma)
    res = run_bass_kernel_spmd(
        nc, in_maps, core_ids=list(range(NCORES)), **spmd_kwargs
    )
    return gather_out(res.results, x), res


def kernel(x, Wf, Wg, Wh, gamma):
    out, _ = run(x, Wf, Wg, Wh, gamma)
    return out


# revision 31
# speedup vs baseline: 1.2270x; 1.0018x over previous
"""Trainium2 Bass kernel for nn_AttentionBlock (B=4, H=W=64, C=512, Cr=64).

Reference computation (per batch sample b):
    xf = x[b].reshape(N=4096, C=512)
    q = xf @ Wf; k = xf @ Wg; v = xf @ Wh
    attn = softmax(q @ k.T, axis=-1)
    out[b] = gamma * (attn @ v) + x[b]

Sharding: 8 cores, data-parallel over B=4 with 2-way sequence-parallel over
query rows. Core c handles batch c//2, query-row half c%2 (2048 rows).
Each core receives the full 4096x512 x of its batch, permuted so its OWN
2048 query rows come first (softmax over keys is permutation invariant as
long as k and v use the same key order, which they do). The program is
identical on all cores (SPMD); only the input data differs.

Host-side marshaling pre-packs every input into the exact SBUF layout
(partition-major, KC-chunked) so each tensor loads with ONE wide DMA.

Per-core dataflow:
  1. qT = Wf.T @ xT (bf16); kT = Wg.T @ xT (bf16, duplicated onto
     partitions 64..127 for PE-array row-packing of the score matmuls);
     v = (16x) @ (16Wh) in fp8e4 DoubleRow, stored as fp8 pair tiles
     v8p[kt/2][128, 2, 512] = 16*v.
  2. Per 512-row block, two passes over the 32 key tiles:
     P1: packed score matmuls (two 64-contraction matmuls on disjoint PE
         halves) -> exp on ScalarE (bf16, no max subtraction: scores are
         fp32/bf16-safe) into a 20-deep e1 ring; row sums accumulate as a
         TRANSPOSED [1, 512] psum row via ones.T @ e1 matmuls (1-column
         stationary => no LDWEIGHTS cost).
     F:  F_row = 1/Z on VectorE, broadcast to all 128 partitions on
         GpSimd (partition_broadcast).
     P2: p8 = e1 * F on VectorE quantized to fp8e4 (p8 <= ~1, exactly
         softmax rows); o += p8.T @ v8 as fp8 DoubleRow matmuls (2 key
         tiles per matmul, 2x PE throughput).
     P1 of block b+1 is interleaved pair-by-pair with P2 of block b so
     ScalarE (exp) and PE stay concurrently busy; the first block's P1
     interleaves with the second half of the v projection.
  3. out = o * (gamma/16) + x fused on VectorE (the /16 compensates the
     fp8 prescales: p8 @ (16 v) = 16 * attn@v), per-row-chunk DMA out.
"""

import sys

if "/opt/trn_rl_repo" not in sys.path:
    sys.path.insert(0, "/opt/trn_rl_repo")

import numpy as np

_BUILt = {}

B, H, W, C = 4, 64, 64, 512
CR = 64          # C // reduction ratio
N = H * W        # 4096 keys per batch
R = N // 2       # 2048 query rows per core
NCORES = 8
NKT = N // 128   # 32 key tiles
NKP = NKT // 2   # 16 key-tile pairs
KC = C // 128    # 4 contraction chunks over C


def _build():
    import ml_dtypes
    import concourse.bass as bass
    import concourse.mybir as mybir
    import concourse.tile as tile
    from concourse import bacc

    f32 = mybir.dt.float32
    bf16 = mybir.dt.bfloat16
    fp8 = mybir.dt.float8e4
    Exp = mybir.ActivationFunctionType.Exp
    mult = mybir.AluOpType.mult
    add = mybir.AluOpType.add
    DR = mybir.MatmulPerfMode.DoubleRow

    nc = bacc.Bacc(
        "TRN2",
        target_bir_lowering=False,
        debug=False,
        num_devices=NCORES,
    )

    xta_d = nc.dram_tensor("xta", [128, KC, R], bf16, kind="ExternalInput")
    xtb_d = nc.dram_tensor("xtb", [128, KC, R], bf16, kind="ExternalInput")
    x8a_d = nc.dram_tensor("x8a", [128, KC, R], fp8, kind="ExternalInput")
    x8b_d = nc.dram_tensor("x8b", [128, KC, R], fp8, kind="ExternalInput")
    wf_d = nc.dram_tensor("wf", [128, KC, CR], bf16, kind="ExternalInput")
    wg_d = nc.dram_tensor("wg", [128, KC, CR], bf16, kind="ExternalInput")
    wh8_d = nc.dram_tensor("wh8", [128, KC, C], fp8, kind="ExternalInput")
    gam_d = nc.dram_tensor("gammav", [128, 1], f32, kind="ExternalInput")
    xres_d = nc.dram_tensor("xres", [128, 16, C], bf16, kind="ExternalInput")
    out_d = nc.dram_tensor("out", [128, 16, C], f32, kind="ExternalOutput")

    ones_d = nc.inline_tensor(
        np.ones((128, 1), dtype=ml_dtypes.bfloat16), name="onesc"
    )

    with tile.TileContext(nc) as tc:
        with (
            tc.tile_pool(name="const", bufs=1) as cpool,
            tc.tile_pool(name="stand", bufs=1) as spool,
            tc.tile_pool(name="e1", bufs=22) as e1_pool,
            tc.tile_pool(name="p8", bufs=4) as p8_pool,
            tc.tile_pool(name="fpool", bufs=2) as f_pool,
            tc.tile_pool(name="small", bufs=8) as sm_pool,
            tc.tile_pool(name="outp", bufs=4) as out_pool,
        ):
            ones_sb = cpool.tile([128, 1], bf16, name="ones_sb")
            gam_sb = cpool.tile([128, 1], f32, name="gam_sb")
            wf_sb = cpool.tile([128, KC, CR], bf16, name="wf_sb")
            wg_sb = cpool.tile([128, KC, CR], bf16, name="wg_sb")
            wh8_sb = cpool.tile([128, KC, C], fp8, name="wh8_sb")

            xta_sb = spool.tile([128, KC, R], bf16, name="xta_sb")
            xtb_sb = spool.tile([128, KC, R], bf16, name="xtb_sb")
            x8a_sb = spool.tile([128, KC, R], fp8, name="x8a_sb")
            x8b_sb = spool.tile([128, KC, R], fp8, name="x8b_sb")
            xres_sb = spool.tile([128, 16, C], bf16, name="xres_sb")
            v8p = [
                spool.tile([128, 2, C], fp8, name=f"v8p{t}")
                for t in range(NKP)
            ]
            kTd = spool.tile([128, N], bf16, name="kTd")
            qTd = spool.tile([128, R], bf16, name="qTd")

            # ---- one wide DMA per tensor; q projection unblocks first ----
            nc.sync.dma_start(out=wf_sb[:], in_=wf_d[:])
            for kc in range(KC):
                nc.sync.dma_start(
                    out=xta_sb[:, kc, :], in_=xta_d[:, kc, :]
                )
            nc.sync.dma_start(out=wg_sb[:], in_=wg_d[:])
            nc.sync.dma_start(out=xtb_sb[:], in_=xtb_d[:])
            nc.sync.dma_start(out=wh8_sb[:], in_=wh8_d[:])
            nc.sync.dma_start(out=x8a_sb[:], in_=x8a_d[:])
            nc.sync.dma_start(out=x8b_sb[:], in_=x8b_d[:])
            nc.sync.dma_start(out=ones_sb[:], in_=ones_d[:])
            nc.sync.dma_start(out=gam_sb[:], in_=gam_d[:])
            nc.sync.dma_start(out=xres_sb[:], in_=xres_d[:])

            with tc.tile_pool(name="psc", bufs=1, space="PSUM") as p3:
                saved_e1 = [[None] * NKP for _ in range(4)]
                f_tiles = [None] * 4
                o_cur = [None]
                o_pool = [None]

                zt_cur = [None]

                def emit_zt(blk, p, zt):
                    e1p = saved_e1[blk][p]
                    for sub in range(2):
                        nc.tensor.matmul(
                            zt[:],
                            lhsT=ones_sb[:],
                            rhs=e1p[:, sub, :],
                            start=(p == 0 and sub == 0),
                            stop=(p == NKP - 1 and sub == 1),
                            skip_group_check=True,
                        )

                def emit_p1_pair(blk, ktp):
                    # zt (row sums) for pair ktp-2 is emitted here, two
                    # pairs behind the exp that feeds it: the PE is
                    # in-order, so a zt matmul gated on a just-issued exp
                    # would stall the PE (and everything queued after it)
                    # on ScalarE for ~half the exp latency every pair.
                    if ktp == 0:
                        zt_cur[0] = p3.tile(
                            [1, 512], f32, tag="zt", name=f"zt{blk}"
                        )
                    e1p = e1_pool.tile(
                        [128, 2, 512], bf16, tag="e1", name="e1"
                    )
                    saved_e1[blk][ktp] = e1p
                    for sub in range(2):
                        kt = 2 * ktp + sub
                        hp = sub * CR
                        sch = p3.tile(
                            [128, 512], f32, tag="sc", bufs=3, name="sch"
                        )
                        nc.tensor.matmul(
                            sch[:],
                            lhsT=kTd[
                                hp : hp + CR, kt * 128 : (kt + 1) * 128
                            ],
                            rhs=qTd[
                                hp : hp + CR, blk * 512 : (blk + 1) * 512
                            ],
                            start=True,
                            stop=True,
                        )
                        nc.scalar.activation(
                            e1p[:, sub, :], sch[:], Exp
                        )
                    # batched: 4 same-shape zt matmuls back-to-back every
                    # other pair (fewer MM type-transitions on the PE, which
                    # each cost ~100ns of pipeline drain), still lagged >= 2
                    # pairs behind their exps
                    if ktp >= 3 and ktp % 2 == 1:
                        emit_zt(blk, ktp - 3, zt_cur[0])
                        emit_zt(blk, ktp - 2, zt_cur[0])
                    if ktp == NKP - 1:
                        emit_zt(blk, ktp - 1, zt_cur[0])
                        emit_zt(blk, ktp, zt_cur[0])
                    return zt_cur[0]

                def emit_f(blk, zt):
                    frow = sm_pool.tile(
                        [1, 512], bf16, tag="frow", name="frow"
                    )
                    with nc.allow_low_precision(
                        reason="1/Z at bf16: 0.4% row-scale noise, far "
                        "under the fp8e4 P quantization already accepted"
                    ):
                        nc.vector.reciprocal(frow[:], zt[:])
                    fsb = f_pool.tile(
                        [128, 2, 512], bf16, tag="fsb", name="fsb"
                    )
                    for sub in range(2):
                        nc.gpsimd.partition_broadcast(
                            fsb[:, sub, :], frow[0:1, :]
                        )
                    f_tiles[blk] = fsb

                def emit_p2_pair(blk, ktp):
                    if ktp == 0:
                        o_cur[0] = [
                            o_pool[0].tile(
                                [128, C], f32, tag=f"o{rc}",
                                name=f"ops{blk}_{rc}",
                            )
                            for rc in range(4)
                        ]
                    e1p = saved_e1[blk][ktp]
                    saved_e1[blk][ktp] = None
                    p8t = p8_pool.tile(
                        [128, 2, 512], fp8, tag="p8", name="p8"
                    )
                    nc.vector.tensor_mul(
                        p8t[:, :, :], e1p[:, :, :], f_tiles[blk][:, :, :]
                    )
                    for rc in range(4):
                        nc.tensor.matmul(
                            o_cur[0][rc][:],
                            lhsT=p8t[:, :, rc * 128 : (rc + 1) * 128],
                            rhs=v8p[ktp][:],
                            start=(ktp == 0),
                            stop=(ktp == NKP - 1),
                            perf_mode=DR,
                        )

                def emit_epilogue(blk):
                    for rc in range(4):
                        ot = out_pool.tile([128, C], f32, tag="ot", name="ot")
                        nc.vector.scalar_tensor_tensor(
                            out=ot[:],
                            in0=o_cur[0][rc][:],
                            scalar=gam_sb[:],
                            in1=xres_sb[:, blk * 4 + rc, :],
                            op0=mult,
                            op1=add,
                        )
                        nc.sync.dma_start(
                            out=out_d[:, blk * 4 + rc, :], in_=ot[:]
                        )

                def emit_kq(w_sb, dst_sb, xT, nt_local, dst_off, kq_pool,
                            on_vector=False):
                    ps = kq_pool.tile([CR, 512], f32, tag="kq", name="kqp")
                    for kc in range(KC):
                        nc.tensor.matmul(
                            ps[:],
                            lhsT=w_sb[:, kc, :],
                            rhs=xT[:, kc, nt_local * 512 : (nt_local + 1) * 512],
                            start=(kc == 0),
                            stop=(kc == KC - 1),
                        )
                    dst = dst_sb[0:CR, dst_off : dst_off + 512]
                    if on_vector:
                        nc.vector.tensor_copy(dst, ps[:])
                    else:
                        nc.scalar.copy(dst, ps[:])

                def emit_v(x8, kt, vps_pool, on_scalar=False):
                    kt16 = kt % 16
                    sl = slice(kt16 * 128, (kt16 + 1) * 128)
                    ps = vps_pool.tile([128, C], f32, tag="vps", name="vp")
                    nc.tensor.matmul(
                        ps[:],
                        lhsT=x8[:, 0:2, sl],
                        rhs=wh8_sb[:, 0:2, :],
                        start=True,
                        stop=False,
                        perf_mode=DR,
                    )
                    nc.tensor.matmul(
                        ps[:],
                        lhsT=x8[:, 2:4, sl],
                        rhs=wh8_sb[:, 2:4, :],
                        start=False,
                        stop=True,
                        perf_mode=DR,
                    )
                    dst = v8p[kt // 2][:, kt % 2, :]
                    if on_scalar:
                        nc.scalar.activation(
                            dst, ps[:], mybir.ActivationFunctionType.Copy,
                            scale=0.0625,
                        )
                    else:
                        nc.vector.tensor_scalar_mul(dst, ps[:], 0.0625)

                # ---- phase 2: projections (q, k fully; v first half) ----
                with tc.tile_pool(name="ps2kq", bufs=3, space="PSUM") as kqp:
                    for nt in range(4):
                        emit_kq(wf_sb, qTd, xta_sb, nt, nt * 512, kqp,
                                on_vector=(nt % 2 == 1))
                    nc.sync.dma_start(out=qTd[CR:128, :], in_=qTd[0:CR, :])
                    for nt in range(4):
                        emit_kq(wg_sb, kTd, xta_sb, nt, nt * 512, kqp,
                                on_vector=(nt % 2 == 1))
                    # split the kT h64-duplication per half so block 0's
                    # first 8 score pairs unblock right after k-a
                    nc.sync.dma_start(
                        out=kTd[CR:128, 0:R], in_=kTd[0:CR, 0:R]
                    )
                    for nt in range(4):
                        emit_kq(wg_sb, kTd, xtb_sb, nt, 2048 + nt * 512, kqp,
                                on_vector=(nt % 2 == 1))
                    nc.sync.dma_start(
                        out=kTd[CR:128, R:N], in_=kTd[0:CR, R:N]
                    )

                with tc.tile_pool(name="ps2v", bufs=4, space="PSUM") as vps:
                    for kt in range(16):
                        emit_v(x8a_sb, kt, vps, on_scalar=(kt % 2 == 1))
                    # ---- head slot: v second half interleaved with P1 of
                    # block 0 (scores need full kT; v8b tiles are consumed
                    # only from P2 of block 0 onwards) ----
                    for p in range(NKP):
                        emit_v(x8b_sb, 16 + p, vps)
                        zt0 = emit_p1_pair(0, p)

                emit_f(0, zt0)
                with tc.tile_pool(name="po", bufs=1, space="PSUM") as pop:
                    o_pool[0] = pop
                    # ---- mid slots: P1(b+1) pair-interleaved with P2(b),
                    # P1 LEADING by 2 pairs: the PE queue is in-order, so
                    # o-matmuls emitted between a score and the exp-gated
                    # reuse of its psum slot would put the o latency inside
                    # the score->exp->ring feedback loop ----
                    LEAD = 2
                    for b in range(3):
                        for p in range(NKP + LEAD):
                            if p < NKP:
                                ztn = emit_p1_pair(b + 1, p)
                            if p >= LEAD:
                                emit_p2_pair(b, p - LEAD)
                        emit_f(b + 1, ztn)
                        emit_epilogue(b)
                    # ---- tail slot ----
                    for p in range(NKP):
                        emit_p2_pair(3, p)
                    emit_epilogue(3)

    nc.compile()
    return nc


def _get_nc():
    if "nc" not in _BUILt:
        _BUILt["nc"] = _build()
    return _BUILt["nc"]


def make_in_maps(x, Wf, Wg, Wh, gamma):
    import ml_dtypes

    bf16 = ml_dtypes.bfloat16
    fp8 = ml_dtypes.float8_e4m3

    def chunkp(a, d):
        # [KC*128, d] -> [128, KC, d] partition-major
        return np.ascontiguousarray(
            a.reshape(KC, 128, d).transpose(1, 0, 2)
        )

    x = np.asarray(x, dtype=np.float32)
    gv = np.full(
        (128, 1), np.float32(np.asarray(gamma).reshape(-1)[0]) / 16.0,
        dtype=np.float32,
    )
    wf = chunkp(np.asarray(Wf, np.float32).astype(bf16), CR)
    wg = chunkp(np.asarray(Wg, np.float32).astype(bf16), CR)
    wh8 = chunkp((np.asarray(Wh, np.float32) * 16.0).astype(fp8), C)
    in_maps = []
    for core in range(NCORES):
        b, h = divmod(core, 2)
        xb = x[b].reshape(N, C)
        own = xb[h * R : (h + 1) * R]
        other = xb[(1 - h) * R : (2 - h) * R]
        xp = np.concatenate([own, other], axis=0)
        xpT = xp.T  # [C, N]
        xt = chunkp(xpT.astype(bf16), N)
        x8 = chunkp((xpT * 16.0).astype(fp8), N)
        xres = np.ascontiguousarray(
            own.reshape(16, 128, C).transpose(1, 0, 2).astype(bf16)
        )
        in_maps.append(
            {
                "xta": np.ascontiguousarray(xt[:, :, 0:R]),
                "xtb": np.ascontiguousarray(xt[:, :, R:N]),
                "x8a": np.ascontiguousarray(x8[:, :, 0:R]),
                "x8b": np.ascontiguousarray(x8[:, :, R:N]),
                "wf": wf,
                "wg": wg,
                "wh8": wh8,
                "gammav": gv,
                "xres": xres,
            }
        )
    return in_maps


def gather_out(results, x):
    out = np.empty((B, N, C), dtype=np.float32)
    for core in range(NCORES):
        b, h = divmod(core, 2)
        o = results[core]["out"].transpose(1, 0, 2).reshape(R, C)
        out[b, h * R : (h + 1) * R] = o
    return out.reshape(B, H, W, C)


def run(x, Wf, Wg, Wh, gamma, **spmd_kwargs):
    from concourse.bass_utils import run_bass_kernel_spmd

    nc = _get_nc()
    in_maps = make_in_maps(x, Wf, Wg, Wh, gamma)
    res = run_bass_kernel_spmd(
        nc, in_maps, core_ids=list(range(NCORES)), **spmd_kwargs
    )
    return gather_out(res.results, x), res


def kernel(x, Wf, Wg, Wh, gamma):
    out, _ = run(x, Wf, Wg, Wh, gamma)
    return out
